# revision 41
# baseline (speedup 1.0000x reference)
"""Two-layer GCN (PyG GCNConv semantics) on 8 Trainium2 NeuronCores.

Strategy (1D graph partitioning, destination-sharded, pack-4 bf16 gather):
  * Nodes sorted by in-degree (desc), padded to 128*8 groups; group g owned
    by core g%8.  Table row t = k*(P*J) + p*J + j.  The gather table packs
    4 consecutive nodes per 512B bf16 row (25088 rows), so an int16 row
    index (dma_gather's native index type) spans the whole node set.
  * All normalization is host-folded: w~ = w * dinv[src] * dinv[dst] is
    written into a per-slot lane-masked weight tensor w4 ([P, SD*4] bf16,
    one nonzero lane of 4 per slot).  Self-loop term dinv^2*x goes through
    the W matmul as a second PSUM-accumulated matmul with a host-transposed
    operand; bias is applied via a ones-row appended to the lhsT.
  * Device per layer: one dma_gather per ~2K edges (128 partition-slots x
    16 slot-columns) pulls packed rows into SBUF; DVE applies w4 (masking
    the 3 wrong lanes), collapses lanes with two contiguous bf16 adds, and
    reduces each destination group with a strided reduce_sum into f32.
  * Aggregation runs before the 64x64 weight matmul; the transform is
    PE transposes + 2 matmuls per group (agg+bias, self) accumulated in
    PSUM, relu on the scalar engine.
  * Layer-2 table is built by one AllGather of the layer-1 bf16 output
    into a Shared DRAM tensor.
"""

import math
import sys

from contextlib import ExitStack

import numpy as np

if "/opt/trn_rl_repo" not in sys.path:
    sys.path.insert(0, "/opt/trn_rl_repo")

import ml_dtypes

BF = ml_dtypes.bfloat16

P = 128   # SBUF partitions
C = 8     # NeuronCores
F = 64    # feature width (in = hidden = out = 64)
TILE_COLS = 36    # slot-columns per gather tile (>= max padded degree)
INSTR_COLS = 16   # slot-columns per dma_gather instruction
WAVE = 8          # groups per transform wave (8*64 = 512 = one PSUM bank)


# ---------------------------------------------------------------------------
# Host-side graph preprocessing (index work, normalization, permutations)
# ---------------------------------------------------------------------------

def _plan(n_nodes, edge_index, edge_feats):
    N = int(n_nodes)
    G0 = math.ceil(N / P)
    G_total = math.ceil(G0 / C) * C
    J = G_total // C
    N_pad = G_total * P
    R = N_pad // 4  # pack-4 table rows
    PJ = P * J

    row = np.asarray(edge_index[0], dtype=np.int64)
    col = np.asarray(edge_index[1], dtype=np.int64)
    w = np.asarray(edge_feats, dtype=np.float32)

    # symmetric-norm degrees incl. self-loop weight 1 (host-folded)
    deg = np.bincount(col, weights=w, minlength=N_pad).astype(np.float32) + 1.0
    deg[N:] = 1.0
    dinv = 1.0 / np.sqrt(deg)

    degc = np.bincount(col, minlength=N_pad)  # integer in-degree
    order = np.argsort(-degc, kind="stable")  # descending
    s_of = np.empty(N_pad, np.int64)
    s_of[order] = np.arange(N_pad)
    g_of = s_of // P
    p_of = s_of % P
    k_of = g_of % C
    j_of = g_of // C
    t_of = k_of * PJ + p_of * J + j_of        # table position per node

    Dg = degc[order[np.arange(G_total) * P]]
    Dhat = Dg[0::C].astype(np.int64)          # [J] real max degree per group
    # batch-uniform padding: groups in a batch share one padded degree Db,
    # so the whole batch reduces with a single strided reduce_sum.
    batches = []  # (j0, j1, o0, o1, Db)
    off = np.zeros(J + 1, np.int64)
    j0 = 0
    o = 0
    while j0 < J:
        Db = max(int(Dhat[j0]), 1)
        nj = max(1, min(TILE_COLS // Db, J - j0))
        j1 = j0 + nj
        for j in range(j0, j1):
            off[j] = o + (j - j0) * Db
        o += nj * Db
        batches.append((j0, j1, int(off[j0]), int(o), Db))
        j0 = j1
    off[J] = o
    SD = int(o)

    # edge slot assignment: sort edges by destination table position
    tdst = t_of[col]
    oE = np.argsort(tdst, kind="stable")
    td = tdst[oE]
    dslot = np.arange(len(td), dtype=np.int64) - np.searchsorted(td, td, "left")
    kk = td // PJ
    rem = td - kk * PJ
    pp = rem // J
    jj = rem - pp * J
    assert np.all(dslot < Dhat[jj]), "edge slot exceeded padded degree"

    wt_e = w[oE] * dinv[row[oE]] * dinv[col[oE]]   # w~ per edge
    tsrc = t_of[row[oE]]
    prow = (tsrc // 4).astype(np.int64)
    lane = (tsrc % 4).astype(np.int64)
    colpos = off[jj] + dslot

    idx = np.zeros((C, P, SD), np.int64)
    w4 = np.zeros((C, P, SD * 4), np.float32)
    idx[kk, pp, colpos] = prow
    w4[kk, pp, colpos * 4 + lane] = wt_e
    idx32 = idx.astype(np.int32)

    # wrapped int16 index layout: flat i = col*128 + p lives at
    # [i % 16, i // 16], replicated 8x down the partitions.
    flat = idx.transpose(0, 2, 1).reshape(C, SD * P)       # [C, i]
    wrap = flat.reshape(C, SD * 8, 16).transpose(0, 2, 1)  # [C, 16, SD*8]
    assert wrap.max() < 2**15
    idx16 = np.tile(wrap, (1, 8, 1)).astype(np.int16)      # [C, 128, SD*8]

    assert max(o1 - o0 for (_, _, o0, o1, _) in batches) <= TILE_COLS

    selfw = (dinv * dinv).astype(np.float32)
    selfw_t = np.zeros(N_pad, np.float32)
    selfw_t[t_of] = selfw                                   # by table pos

    return dict(N=N, N_pad=N_pad, J=J, R=R, PJ=PJ, SD=SD, Dhat=Dhat, off=off,
                t_of=t_of, idx16=idx16, idx32=idx32, w4=w4.astype(BF),
                selfw_t=selfw_t, batches=batches)


def _make_in_maps(plan, node_feats, W1, b1, W2, b2):
    N, N_pad, J, R, PJ = (plan[k] for k in ("N", "N_pad", "J", "R", "PJ"))
    x_perm = np.zeros((N_pad, F), np.float32)
    x_perm[plan["t_of"][:N]] = np.asarray(node_feats, np.float32)
    table1 = np.ascontiguousarray(x_perm.astype(BF).reshape(R, 4 * F))

    sx1 = plan["selfw_t"][:, None] * x_perm                 # [N_pad(t), F]
    sw2 = np.ascontiguousarray(plan["selfw_t"].reshape(C, P, J))

    Wb1 = np.ascontiguousarray(np.vstack(
        [np.asarray(W1, np.float32), np.asarray(b1, np.float32)[None, :]]))
    Wb2 = np.ascontiguousarray(np.vstack(
        [np.asarray(W2, np.float32), np.asarray(b2, np.float32)[None, :]]))
    ident = np.eye(P, dtype=np.float32)

    in_maps = []
    for k in range(C):
        sxk = sx1[k * PJ:(k + 1) * PJ].reshape(P, J, F)
        sxT = np.ascontiguousarray(
            sxk.transpose(2, 1, 0).reshape(F, J * P))       # [64, J*128]
        in_maps.append({
            "table1": table1,
            "idx16": np.ascontiguousarray(plan["idx16"][k]),
            "idx32": np.ascontiguousarray(plan["idx32"][k]),
            "w4": np.ascontiguousarray(plan["w4"][k]),
            "sxT": sxT,
            "sw2": np.ascontiguousarray(sw2[k]),
            "Wb1": Wb1, "Wb2": Wb2, "ident": ident,
        })
    return in_maps


def _unshard(plan, outs):
    J, N = plan["J"], plan["N"]
    full = np.concatenate(
        [o.reshape(P, J, F).reshape(P * J, F) for o in outs], axis=0)
    return np.ascontiguousarray(full[plan["t_of"][:N]])


# ---------------------------------------------------------------------------
# Device program
# ---------------------------------------------------------------------------

def _build(plan):
    from concourse import bacc, bass, mybir
    import concourse.tile as tile
    from concourse.library_config import mlp

    import os
    debug_mode = os.environ.get("KERNEL_DEBUG", "")

    f32 = mybir.dt.float32
    bf16 = mybir.dt.bfloat16
    i16 = mybir.dt.int16
    i32 = mybir.dt.int32
    J, R, PJ, SD = plan["J"], plan["R"], plan["PJ"], plan["SD"]
    Dhat, off, batches = plan["Dhat"], plan["off"], plan["batches"]

    n_queues = int(os.environ.get("KERNEL_QUEUES", "4"))
    scratch = int(os.environ.get("KERNEL_SCRATCH", "49152"))
    nc = bacc.Bacc(None, target_bir_lowering=False, num_devices=C,
                   num_swdge_queues=n_queues,
                   dynamic_dma_scratch_size=scratch)

    tab1_in = nc.dram_tensor("table1", [R, 4 * F], bf16, kind="ExternalInput")
    idx_in = nc.dram_tensor("idx16", [P, SD * 8], i16, kind="ExternalInput")
    idx32_in = nc.dram_tensor("idx32", [P, SD], i32, kind="ExternalInput")
    w4_in = nc.dram_tensor("w4", [P, SD * 4], bf16, kind="ExternalInput")
    sxT_in = nc.dram_tensor("sxT", [F, J * P], f32, kind="ExternalInput")
    sw2_in = nc.dram_tensor("sw2", [P, J], f32, kind="ExternalInput")
    Wb1_in = nc.dram_tensor("Wb1", [F + 1, F], f32, kind="ExternalInput")
    Wb2_in = nc.dram_tensor("Wb2", [F + 1, F], f32, kind="ExternalInput")
    id_in = nc.dram_tensor("ident", [P, P], f32, kind="ExternalInput")
    out_t = nc.dram_tensor("out", [P, J * F], f32, kind="ExternalOutput")

    ag2 = nc.dram_tensor("ag_in2", [PJ, F], bf16)
    table2 = nc.dram_tensor("table2", [C * PJ, F], bf16)

    groups = [list(range(C))]

    with ExitStack() as ctx:
        tc = ctx.enter_context(tile.TileContext(nc))
        big = ctx.enter_context(tc.tile_pool(name="big", bufs=1))
        gp = ctx.enter_context(tc.tile_pool(name="gp", bufs=3))
        t2p = ctx.enter_context(tc.tile_pool(name="t2p", bufs=2))
        wp = ctx.enter_context(tc.tile_pool(name="wp", bufs=2))
        pT = ctx.enter_context(tc.tile_pool(name="pT", bufs=2, space="PSUM"))
        pZ = ctx.enter_context(tc.tile_pool(name="pZ", bufs=2, space="PSUM"))

        idxt = big.tile([P, SD * 8], i16)
        if "ind" in debug_mode:
            idx32t = big.tile([P, SD], i32)
            nc.sync.dma_start(out=idx32t[:], in_=idx32_in[:, :])
        w4t = big.tile([P, SD * 4], bf16)
        sw2t = big.tile([P, J], f32)
        Wb1t = big.tile([F + 1, F], f32)
        Wb2t = big.tile([F + 1, F], f32)
        identt = big.tile([P, P], f32)
        agg = big.tile([P, J * F], f32)
        zbf = big.tile([P, J * F], bf16)
        aggTt = big.tile([F + 1, WAVE * P], f32)
        sfxTt = big.tile([F, WAVE * P], f32)

        # ---- loads / init ----
        if "ind" not in debug_mode:
            nc.gpsimd.load_library(mlp)
        nc.sync.dma_start(out=idxt[:], in_=idx_in[:, :])
        nc.sync.dma_start(out=w4t[:], in_=w4_in[:, :])
        nc.sync.dma_start(out=sw2t[:], in_=sw2_in[:, :])
        nc.sync.dma_start(out=Wb1t[:], in_=Wb1_in[:, :])
        nc.sync.dma_start(out=Wb2t[:], in_=Wb2_in[:, :])
        nc.sync.dma_start(out=identt[:], in_=id_in[:, :])
        nc.vector.memset(aggTt[F:F + 1, :], 1.0)  # bias ones-row

        qrr = [0]  # round-robin SWDGE queue cursor

        def aggregate(table_ap):
            for (j0, j1, o0, o1, Db) in batches:
                S = o1 - o0
                g = gp.tile([P, TILE_COLS * 4 * F], bf16, tag="g")
                if "ind" in debug_mode:
                    for d in range(S):
                        nc.gpsimd.indirect_dma_start(
                            out=g[:, d * 256:(d + 1) * 256],
                            out_offset=None,
                            in_=table_ap,
                            in_offset=bass.IndirectOffsetOnAxis(
                                ap=idx32t[:, o0 + d:o0 + d + 1], axis=0),
                        )
                else:
                    icols = int(os.environ.get("KERNEL_ICOLS", INSTR_COLS))
                    ca = o0
                    while ca < o1:
                        cb = min(ca + icols, o1)
                        n = (cb - ca) * P
                        nc.gpsimd.dma_gather(
                            g[:, (ca - o0) * 256:(cb - o0) * 256].rearrange(
                                "p (s e) -> p s e", e=256),
                            table_ap,
                            idxt[:, ca * 8:cb * 8],
                            n, n, 256,
                            queue_num=qrr[0] % n_queues,
                        )
                        qrr[0] += 1
                        ca = cb
                # mask-multiply by w~ + lane collapse, per sub-chunk so DVE
                # overlaps the in-flight gathers of the same batch.  The
                # first collapse add runs in place on g's lane-01 half.
                t2 = t2p.tile([P, TILE_COLS * 64], bf16, tag="t2")
                sub = 2 * INSTR_COLS
                sa = 0
                while sa < S:
                    sb = min(sa + sub, S)
                    ns = sb - sa
                    gv = g[:, sa * 256:sb * 256].rearrange(
                        "p (c f) -> p c f", f=F)
                    nc.vector.tensor_tensor(
                        out=gv, in0=gv,
                        in1=w4t[:, (o0 + sa) * 4:(o0 + sb) * 4].unsqueeze(
                            2).to_broadcast([P, ns * 4, F]),
                        op=mybir.AluOpType.mult,
                    )
                    ge = g[:, sa * 256:sb * 256].rearrange(
                        "p (s e) -> p s e", e=256)
                    nc.vector.tensor_tensor(
                        out=ge[:, :, 0:128],
                        in0=ge[:, :, 0:128], in1=ge[:, :, 128:256],
                        op=mybir.AluOpType.add,
                    )
                    nc.vector.tensor_tensor(
                        out=t2[:, sa * 64:sb * 64].rearrange(
                            "p (s e) -> p s e", e=64),
                        in0=ge[:, :, 0:64], in1=ge[:, :, 64:128],
                        op=mybir.AluOpType.add,
                    )
                    sa = sb
                # one strided reduce for the whole batch (uniform Db)
                nc.vector.reduce_sum(
                    out=agg[:, j0 * F:j1 * F],
                    in_=t2[:, :S * F].rearrange(
                        "p (j d f) -> p j f d", d=Db, f=F),
                    axis=mybir.AxisListType.X,
                )

        def transform(Wbt, layer):
            for wi, w0 in enumerate(range(0, J, WAVE)):
                w1 = min(w0 + WAVE, J)
                nW = w1 - w0
                aT = aggTt
                if layer == 1:
                    sxw = wp.tile([F, WAVE * P], f32, tag="sxw")
                    nc.sync.dma_start(out=sxw[:, :nW * P],
                                      in_=sxT_in[:, w0 * P:w1 * P])
                    sT = sxw
                else:
                    sfx = wp.tile([P, WAVE * F], f32, tag="sfx")
                    nc.vector.tensor_tensor(
                        out=sfx[:, :nW * F].rearrange("p (j f) -> p j f", f=F),
                        in0=zbf[:, w0 * F:w1 * F].rearrange(
                            "p (j f) -> p j f", f=F),
                        in1=sw2t[:, w0:w1].unsqueeze(2).to_broadcast(
                            [P, nW, F]),
                        op=mybir.AluOpType.mult,
                    )
                    sT = sfxTt
                nhalf = math.ceil(nW / 4)
                for h in range(nhalf):
                    lo = w0 + h * 4
                    hi = min(lo + 4, w1)
                    nn = hi - lo
                    psT = pT.tile([F, 4 * P], f32, tag="pT")
                    for i, j in enumerate(range(lo, hi)):
                        nc.tensor.transpose(
                            out=psT[:, i * P:(i + 1) * P],
                            in_=agg[:, j * F:(j + 1) * F],
                            identity=identt[:],
                        )
                    nc.scalar.copy(
                        out=aT[0:F, (h * 4) * P:(h * 4 + nn) * P],
                        in_=psT[:, :nn * P])
                    if layer == 2:
                        psS = pT.tile([F, 4 * P], f32, tag="pS")
                        for i, j in enumerate(range(lo, hi)):
                            jj = j - w0
                            nc.tensor.transpose(
                                out=psS[:, i * P:(i + 1) * P],
                                in_=sfx[:, jj * F:(jj + 1) * F],
                                identity=identt[:],
                            )
                        nc.scalar.copy(
                            out=sT[0:F, (h * 4) * P:(h * 4 + nn) * P],
                            in_=psS[:, :nn * P])
                psZ = pZ.tile([P, WAVE * F], f32, tag="pZ")
                for i in range(nW):
                    nc.tensor.matmul(
                        out=psZ[:, i * F:(i + 1) * F],
                        lhsT=aT[:, i * P:(i + 1) * P],
                        rhs=Wbt[:, :],
                        start=True, stop=False,
                    )
                    nc.tensor.matmul(
                        out=psZ[:, i * F:(i + 1) * F],
                        lhsT=sT[0:F, i * P:(i + 1) * P],
                        rhs=Wbt[0:F, :],
                        start=False, stop=True,
                    )
                if layer == 1:
                    nc.scalar.activation(
                        out=zbf[:, w0 * F:w1 * F],
                        in_=psZ[:, :nW * F],
                        func=mybir.ActivationFunctionType.Relu,
                    )
                else:
                    ot = wp.tile([P, WAVE * F], f32, tag="ot")
                    nc.scalar.activation(
                        out=ot[:, :nW * F],
                        in_=psZ[:, :nW * F],
                        func=mybir.ActivationFunctionType.Relu,
                    )
                    nc.sync.dma_start(out=out_t[:, w0 * F:w1 * F],
                                      in_=ot[:, :nW * F])

        import os
        debug_mode = os.environ.get("KERNEL_DEBUG", "")

        # ---- layer 1 ----
        with nc.named_scope("agg1"):
            aggregate(tab1_in[:, :])
        with nc.named_scope("xform1"):
            transform(Wb1t, 1)
        if debug_mode == "l1":
            # debug: emit layer-1 activations as the output, skip the rest
            nc.vector.tensor_copy(out=agg[:], in_=zbf[:])
            nc.sync.dma_start(out=out_t[:, :], in_=agg[:])
        else:
            with nc.named_scope("allgather2"):
                ag2_ap = ag2.ap().rearrange("(p j) f -> p (j f)", p=P)
                nc.sync.dma_start(out=ag2_ap, in_=zbf[:])
                nc.gpsimd.collective_compute(
                    "AllGather", mybir.AluOpType.bypass, replica_groups=groups,
                    ins=[ag2.ap().opt()], outs=[table2.ap().opt()],
                )

            # ---- layer 2 ----
            tab2_ap = table2.ap().rearrange("(r q) f -> r (q f)", q=4)
            with nc.named_scope("agg2"):
                aggregate(tab2_ap)
            with nc.named_scope("xform2"):
                transform(Wb2t, 2)

    nc.compile()
    return nc


# ---------------------------------------------------------------------------
# Entry point
# ---------------------------------------------------------------------------

LAST_RESULT = None  # BassKernelResults of the most recent kernel() call


def kernel(node_feats, edge_index, edge_feats, W1, b1, W2, b2):
    global LAST_RESULT
    from concourse.bass_utils import run_bass_kernel_spmd

    plan = _plan(node_feats.shape[0], edge_index, edge_feats)
    nc = _build(plan)
    in_maps = _make_in_maps(plan, node_feats, W1, b1, W2, b2)
    res = run_bass_kernel_spmd(nc, in_maps, core_ids=list(range(C)))
    LAST_RESULT = res
    return _unshard(plan, [res.results[k]["out"] for k in range(C)])


# revision 42
# speedup vs baseline: 1.0729x; 1.0729x over previous
"""Two-layer GCN (PyG GCNConv semantics) on 8 Trainium2 NeuronCores.

Strategy (1D graph partitioning, destination-sharded, pack-4 bf16 gather):
  * Nodes sorted by in-degree (desc), padded to 128*8 groups; group g owned
    by core g%8.  Table row t = k*(P*J) + p*J + j.  The gather table packs
    4 consecutive nodes per 512B bf16 row (25088 rows), so an int16 row
    index (dma_gather's native index type) spans the whole node set.
  * All normalization is host-folded: w~ = w * dinv[src] * dinv[dst] is
    written into a per-slot lane-masked weight tensor w4 ([P, SD*4] bf16,
    one nonzero lane of 4 per slot).  Self-loop term dinv^2*x goes through
    the W matmul as a second PSUM-accumulated matmul with a host-transposed
    operand; bias is applied via a ones-row appended to the lhsT.
  * Device per layer: one dma_gather per ~2K edges (128 partition-slots x
    16 slot-columns) pulls packed rows into SBUF; DVE applies w4 (masking
    the 3 wrong lanes), collapses lanes with two contiguous bf16 adds, and
    reduces each destination group with a strided reduce_sum into f32.
  * Aggregation runs before the 64x64 weight matmul; the transform is
    PE transposes + 2 matmuls per group (agg+bias, self) accumulated in
    PSUM, relu on the scalar engine.
  * Layer-2 table is built by one AllGather of the layer-1 bf16 output
    into a Shared DRAM tensor.
"""

import math
import sys

from contextlib import ExitStack

import numpy as np

if "/opt/trn_rl_repo" not in sys.path:
    sys.path.insert(0, "/opt/trn_rl_repo")

import ml_dtypes

BF = ml_dtypes.bfloat16

P = 128   # SBUF partitions
C = 8     # NeuronCores
F = 64    # feature width (in = hidden = out = 64)
TILE_COLS = 36    # slot-columns per gather tile (>= max padded degree)
INSTR_COLS = 8    # slot-columns per dma_gather instruction (1024 idxs;
                  # >1024 idxs per instruction crashes the gather ucode)
WAVE = 8          # groups per transform wave (8*64 = 512 = one PSUM bank)


# ---------------------------------------------------------------------------
# Host-side graph preprocessing (index work, normalization, permutations)
# ---------------------------------------------------------------------------

def _plan(n_nodes, edge_index, edge_feats):
    N = int(n_nodes)
    G0 = math.ceil(N / P)
    G_total = math.ceil(G0 / C) * C
    J = G_total // C
    N_pad = G_total * P
    R = N_pad // 4  # pack-4 table rows
    PJ = P * J

    row = np.asarray(edge_index[0], dtype=np.int64)
    col = np.asarray(edge_index[1], dtype=np.int64)
    w = np.asarray(edge_feats, dtype=np.float32)

    # symmetric-norm degrees incl. self-loop weight 1 (host-folded)
    deg = np.bincount(col, weights=w, minlength=N_pad).astype(np.float32) + 1.0
    deg[N:] = 1.0
    dinv = 1.0 / np.sqrt(deg)

    degc = np.bincount(col, minlength=N_pad)  # integer in-degree
    order = np.argsort(-degc, kind="stable")  # descending
    s_of = np.empty(N_pad, np.int64)
    s_of[order] = np.arange(N_pad)
    g_of = s_of // P
    p_of = s_of % P
    k_of = g_of % C
    j_of = g_of // C
    t_of = k_of * PJ + p_of * J + j_of        # table position per node

    Dg = degc[order[np.arange(G_total) * P]]
    Dhat = Dg[0::C].astype(np.int64)          # [J] real max degree per group
    # batch-uniform padding: groups in a batch share one padded degree Db,
    # so the whole batch reduces with a single strided reduce_sum.
    batches = []  # (j0, j1, o0, o1, Db)
    off = np.zeros(J + 1, np.int64)
    j0 = 0
    o = 0
    while j0 < J:
        Db = max(int(Dhat[j0]), 1)
        nj = max(1, min(TILE_COLS // Db, J - j0))
        j1 = j0 + nj
        for j in range(j0, j1):
            off[j] = o + (j - j0) * Db
        o += nj * Db
        batches.append((j0, j1, int(off[j0]), int(o), Db))
        j0 = j1
    off[J] = o
    SD = int(o)

    # edge slot assignment: sort edges by destination table position
    tdst = t_of[col]
    oE = np.argsort(tdst, kind="stable")
    td = tdst[oE]
    dslot = np.arange(len(td), dtype=np.int64) - np.searchsorted(td, td, "left")
    kk = td // PJ
    rem = td - kk * PJ
    pp = rem // J
    jj = rem - pp * J
    assert np.all(dslot < Dhat[jj]), "edge slot exceeded padded degree"

    wt_e = w[oE] * dinv[row[oE]] * dinv[col[oE]]   # w~ per edge
    tsrc = t_of[row[oE]]
    prow = (tsrc // 4).astype(np.int64)
    lane = (tsrc % 4).astype(np.int64)
    colpos = off[jj] + dslot

    idx = np.zeros((C, P, SD), np.int64)
    w4 = np.zeros((C, P, SD * 4), np.float32)
    idx[kk, pp, colpos] = prow
    w4[kk, pp, colpos * 4 + lane] = wt_e
    idx32 = idx.astype(np.int32)

    # wrapped int16 index layout: flat i = col*128 + p lives at
    # [i % 16, i // 16], replicated 8x down the partitions.
    flat = idx.transpose(0, 2, 1).reshape(C, SD * P)       # [C, i]
    wrap = flat.reshape(C, SD * 8, 16).transpose(0, 2, 1)  # [C, 16, SD*8]
    assert wrap.max() < 2**15
    idx16 = np.tile(wrap, (1, 8, 1)).astype(np.int16)      # [C, 128, SD*8]

    assert max(o1 - o0 for (_, _, o0, o1, _) in batches) <= TILE_COLS

    selfw = (dinv * dinv).astype(np.float32)
    selfw_t = np.zeros(N_pad, np.float32)
    selfw_t[t_of] = selfw                                   # by table pos

    return dict(N=N, N_pad=N_pad, J=J, R=R, PJ=PJ, SD=SD, Dhat=Dhat, off=off,
                t_of=t_of, idx16=idx16, idx32=idx32, w4=w4.astype(BF),
                selfw_t=selfw_t, batches=batches)


def _make_in_maps(plan, node_feats, W1, b1, W2, b2):
    N, N_pad, J, R, PJ = (plan[k] for k in ("N", "N_pad", "J", "R", "PJ"))
    x_perm = np.zeros((N_pad, F), np.float32)
    x_perm[plan["t_of"][:N]] = np.asarray(node_feats, np.float32)
    table1 = np.ascontiguousarray(x_perm.astype(BF).reshape(R, 4 * F))

    sx1 = plan["selfw_t"][:, None] * x_perm                 # [N_pad(t), F]
    sw2 = np.ascontiguousarray(plan["selfw_t"].reshape(C, P, J))

    Wb1 = np.ascontiguousarray(np.vstack(
        [np.asarray(W1, np.float32), np.asarray(b1, np.float32)[None, :]]))
    Wb2 = np.ascontiguousarray(np.vstack(
        [np.asarray(W2, np.float32), np.asarray(b2, np.float32)[None, :]]))
    ident = np.eye(P, dtype=np.float32)

    in_maps = []
    for k in range(C):
        sxk = sx1[k * PJ:(k + 1) * PJ].reshape(P, J, F)
        sxT = np.ascontiguousarray(
            sxk.transpose(2, 1, 0).reshape(F, J * P))       # [64, J*128]
        in_maps.append({
            "table1": table1,
            "idx16": np.ascontiguousarray(plan["idx16"][k]),
            "idx32": np.ascontiguousarray(plan["idx32"][k]),
            "w4": np.ascontiguousarray(plan["w4"][k]),
            "sxT": sxT,
            "sw2": np.ascontiguousarray(sw2[k]),
            "Wb1": Wb1, "Wb2": Wb2, "ident": ident,
        })
    return in_maps


def _unshard(plan, outs):
    J, N = plan["J"], plan["N"]
    full = np.concatenate(
        [o.reshape(P, J, F).reshape(P * J, F) for o in outs], axis=0)
    return np.ascontiguousarray(full[plan["t_of"][:N]])


# ---------------------------------------------------------------------------
# Device program
# ---------------------------------------------------------------------------

def _build(plan):
    from concourse import bacc, bass, mybir
    import concourse.tile as tile
    from concourse.library_config import mlp

    import os
    debug_mode = os.environ.get("KERNEL_DEBUG", "")

    f32 = mybir.dt.float32
    bf16 = mybir.dt.bfloat16
    i16 = mybir.dt.int16
    i32 = mybir.dt.int32
    J, R, PJ, SD = plan["J"], plan["R"], plan["PJ"], plan["SD"]
    Dhat, off, batches = plan["Dhat"], plan["off"], plan["batches"]

    n_queues = int(os.environ.get("KERNEL_QUEUES", "4"))
    scratch = int(os.environ.get("KERNEL_SCRATCH", "49152"))
    nc = bacc.Bacc(None, target_bir_lowering=False, num_devices=C,
                   num_swdge_queues=n_queues,
                   dynamic_dma_scratch_size=scratch)

    tab1_in = nc.dram_tensor("table1", [R, 4 * F], bf16, kind="ExternalInput")
    idx_in = nc.dram_tensor("idx16", [P, SD * 8], i16, kind="ExternalInput")
    idx32_in = nc.dram_tensor("idx32", [P, SD], i32, kind="ExternalInput")
    w4_in = nc.dram_tensor("w4", [P, SD * 4], bf16, kind="ExternalInput")
    sxT_in = nc.dram_tensor("sxT", [F, J * P], f32, kind="ExternalInput")
    sw2_in = nc.dram_tensor("sw2", [P, J], f32, kind="ExternalInput")
    Wb1_in = nc.dram_tensor("Wb1", [F + 1, F], f32, kind="ExternalInput")
    Wb2_in = nc.dram_tensor("Wb2", [F + 1, F], f32, kind="ExternalInput")
    id_in = nc.dram_tensor("ident", [P, P], f32, kind="ExternalInput")
    out_t = nc.dram_tensor("out", [P, J * F], f32, kind="ExternalOutput")

    ag2 = nc.dram_tensor("ag_in2", [PJ, F], bf16)
    table2 = nc.dram_tensor("table2", [C * PJ, F], bf16)

    groups = [list(range(C))]

    with ExitStack() as ctx:
        tc = ctx.enter_context(tile.TileContext(nc))
        big = ctx.enter_context(tc.tile_pool(name="big", bufs=1))
        gp = ctx.enter_context(tc.tile_pool(name="gp", bufs=3))
        t2p = ctx.enter_context(tc.tile_pool(name="t2p", bufs=2))
        wp = ctx.enter_context(tc.tile_pool(name="wp", bufs=2))
        pT = ctx.enter_context(tc.tile_pool(name="pT", bufs=2, space="PSUM"))
        pZ = ctx.enter_context(tc.tile_pool(name="pZ", bufs=2, space="PSUM"))

        idxt = big.tile([P, SD * 8], i16)
        if "ind" in debug_mode:
            idx32t = big.tile([P, SD], i32)
            nc.sync.dma_start(out=idx32t[:], in_=idx32_in[:, :])
        w4t = big.tile([P, SD * 4], bf16)
        sw2t = big.tile([P, J], f32)
        Wb1t = big.tile([F + 1, F], f32)
        Wb2t = big.tile([F + 1, F], f32)
        identt = big.tile([P, P], f32)
        agg = big.tile([P, J * F], f32)
        zbf = big.tile([P, J * F], bf16)
        aggTt = big.tile([F + 1, WAVE * P], f32)
        sfxTt = big.tile([F, WAVE * P], f32)

        # ---- loads / init ----
        if "ind" not in debug_mode:
            nc.gpsimd.load_library(mlp)
        nc.sync.dma_start(out=idxt[:], in_=idx_in[:, :])
        nc.sync.dma_start(out=w4t[:], in_=w4_in[:, :])
        nc.sync.dma_start(out=sw2t[:], in_=sw2_in[:, :])
        nc.sync.dma_start(out=Wb1t[:], in_=Wb1_in[:, :])
        nc.sync.dma_start(out=Wb2t[:], in_=Wb2_in[:, :])
        nc.sync.dma_start(out=identt[:], in_=id_in[:, :])
        nc.vector.memset(aggTt[F:F + 1, :], 1.0)  # bias ones-row

        qrr = [0]  # round-robin SWDGE queue cursor

        def aggregate(table_ap):
            for (j0, j1, o0, o1, Db) in batches:
                S = o1 - o0
                g = gp.tile([P, TILE_COLS * 4 * F], bf16, tag="g")
                if "ind" in debug_mode:
                    for d in range(S):
                        nc.gpsimd.indirect_dma_start(
                            out=g[:, d * 256:(d + 1) * 256],
                            out_offset=None,
                            in_=table_ap,
                            in_offset=bass.IndirectOffsetOnAxis(
                                ap=idx32t[:, o0 + d:o0 + d + 1], axis=0),
                        )
                else:
                    icols = int(os.environ.get("KERNEL_ICOLS", INSTR_COLS))
                    ca = o0
                    while ca < o1:
                        cb = min(ca + icols, o1)
                        n = (cb - ca) * P
                        nc.gpsimd.dma_gather(
                            g[:, (ca - o0) * 256:(cb - o0) * 256].rearrange(
                                "p (s e) -> p s e", e=256),
                            table_ap,
                            idxt[:, ca * 8:cb * 8],
                            n, n, 256,
                            queue_num=qrr[0] % n_queues,
                        )
                        qrr[0] += 1
                        ca = cb
                # mask-multiply by w~ + lane collapse, per sub-chunk so DVE
                # overlaps the in-flight gathers of the same batch.  The
                # first collapse add runs in place on g's lane-01 half.
                t2 = t2p.tile([P, TILE_COLS * 64], bf16, tag="t2")
                sub = 2 * INSTR_COLS
                sa = 0
                while sa < S:
                    sb = min(sa + sub, S)
                    ns = sb - sa
                    gv = g[:, sa * 256:sb * 256].rearrange(
                        "p (c f) -> p c f", f=F)
                    nc.vector.tensor_tensor(
                        out=gv, in0=gv,
                        in1=w4t[:, (o0 + sa) * 4:(o0 + sb) * 4].unsqueeze(
                            2).to_broadcast([P, ns * 4, F]),
                        op=mybir.AluOpType.mult,
                    )
                    ge = g[:, sa * 256:sb * 256].rearrange(
                        "p (s e) -> p s e", e=256)
                    nc.vector.tensor_tensor(
                        out=ge[:, :, 0:128],
                        in0=ge[:, :, 0:128], in1=ge[:, :, 128:256],
                        op=mybir.AluOpType.add,
                    )
                    nc.vector.tensor_tensor(
                        out=t2[:, sa * 64:sb * 64].rearrange(
                            "p (s e) -> p s e", e=64),
                        in0=ge[:, :, 0:64], in1=ge[:, :, 64:128],
                        op=mybir.AluOpType.add,
                    )
                    sa = sb
                # one strided reduce for the whole batch (uniform Db)
                nc.vector.reduce_sum(
                    out=agg[:, j0 * F:j1 * F],
                    in_=t2[:, :S * F].rearrange(
                        "p (j d f) -> p j f d", d=Db, f=F),
                    axis=mybir.AxisListType.X,
                )

        def transform(Wbt, layer):
            for wi, w0 in enumerate(range(0, J, WAVE)):
                w1 = min(w0 + WAVE, J)
                nW = w1 - w0
                aT = aggTt
                if layer == 1:
                    sxw = wp.tile([F, WAVE * P], f32, tag="sxw")
                    nc.sync.dma_start(out=sxw[:, :nW * P],
                                      in_=sxT_in[:, w0 * P:w1 * P])
                    sT = sxw
                else:
                    sfx = wp.tile([P, WAVE * F], f32, tag="sfx")
                    nc.vector.tensor_tensor(
                        out=sfx[:, :nW * F].rearrange("p (j f) -> p j f", f=F),
                        in0=zbf[:, w0 * F:w1 * F].rearrange(
                            "p (j f) -> p j f", f=F),
                        in1=sw2t[:, w0:w1].unsqueeze(2).to_broadcast(
                            [P, nW, F]),
                        op=mybir.AluOpType.mult,
                    )
                    sT = sfxTt
                nhalf = math.ceil(nW / 4)
                for h in range(nhalf):
                    lo = w0 + h * 4
                    hi = min(lo + 4, w1)
                    nn = hi - lo
                    psT = pT.tile([F, 4 * P], f32, tag="pT")
                    for i, j in enumerate(range(lo, hi)):
                        nc.tensor.transpose(
                            out=psT[:, i * P:(i + 1) * P],
                            in_=agg[:, j * F:(j + 1) * F],
                            identity=identt[:],
                        )
                    nc.scalar.copy(
                        out=aT[0:F, (h * 4) * P:(h * 4 + nn) * P],
                        in_=psT[:, :nn * P])
                    if layer == 2:
                        psS = pT.tile([F, 4 * P], f32, tag="pS")
                        for i, j in enumerate(range(lo, hi)):
                            jj = j - w0
                            nc.tensor.transpose(
                                out=psS[:, i * P:(i + 1) * P],
                                in_=sfx[:, jj * F:(jj + 1) * F],
                                identity=identt[:],
                            )
                        nc.scalar.copy(
                            out=sT[0:F, (h * 4) * P:(h * 4 + nn) * P],
                            in_=psS[:, :nn * P])
                psZ = pZ.tile([P, WAVE * F], f32, tag="pZ")
                for i in range(nW):
                    nc.tensor.matmul(
                        out=psZ[:, i * F:(i + 1) * F],
                        lhsT=aT[:, i * P:(i + 1) * P],
                        rhs=Wbt[:, :],
                        start=True, stop=False,
                    )
                    nc.tensor.matmul(
                        out=psZ[:, i * F:(i + 1) * F],
                        lhsT=sT[0:F, i * P:(i + 1) * P],
                        rhs=Wbt[0:F, :],
                        start=False, stop=True,
                    )
                if layer == 1:
                    nc.scalar.activation(
                        out=zbf[:, w0 * F:w1 * F],
                        in_=psZ[:, :nW * F],
                        func=mybir.ActivationFunctionType.Relu,
                    )
                else:
                    ot = wp.tile([P, WAVE * F], f32, tag="ot")
                    nc.scalar.activation(
                        out=ot[:, :nW * F],
                        in_=psZ[:, :nW * F],
                        func=mybir.ActivationFunctionType.Relu,
                    )
                    nc.sync.dma_start(out=out_t[:, w0 * F:w1 * F],
                                      in_=ot[:, :nW * F])

        import os
        debug_mode = os.environ.get("KERNEL_DEBUG", "")

        # ---- layer 1 ----
        with nc.named_scope("agg1"):
            aggregate(tab1_in[:, :])
        with nc.named_scope("xform1"):
            transform(Wb1t, 1)
        if debug_mode == "l1":
            # debug: emit layer-1 activations as the output, skip the rest
            nc.vector.tensor_copy(out=agg[:], in_=zbf[:])
            nc.sync.dma_start(out=out_t[:, :], in_=agg[:])
        else:
            with nc.named_scope("allgather2"):
                ag2_ap = ag2.ap().rearrange("(p j) f -> p (j f)", p=P)
                nc.sync.dma_start(out=ag2_ap, in_=zbf[:])
                nc.gpsimd.collective_compute(
                    "AllGather", mybir.AluOpType.bypass, replica_groups=groups,
                    ins=[ag2.ap().opt()], outs=[table2.ap().opt()],
                )

            # ---- layer 2 ----
            tab2_ap = table2.ap().rearrange("(r q) f -> r (q f)", q=4)
            with nc.named_scope("agg2"):
                aggregate(tab2_ap)
            with nc.named_scope("xform2"):
                transform(Wb2t, 2)

    nc.compile()
    return nc


# ---------------------------------------------------------------------------
# Entry point
# ---------------------------------------------------------------------------

LAST_RESULT = None  # BassKernelResults of the most recent kernel() call


def kernel(node_feats, edge_index, edge_feats, W1, b1, W2, b2):
    global LAST_RESULT
    from concourse.bass_utils import run_bass_kernel_spmd

    plan = _plan(node_feats.shape[0], edge_index, edge_feats)
    nc = _build(plan)
    in_maps = _make_in_maps(plan, node_feats, W1, b1, W2, b2)
    res = run_bass_kernel_spmd(nc, in_maps, core_ids=list(range(C)))
    LAST_RESULT = res
    return _unshard(plan, [res.results[k]["out"] for k in range(C)])
